# revision 2
# baseline (speedup 1.0000x reference)
"""Bass/Trainium2 kernel for nn_MAC_30554397344312 (gnn_message_passing).

Reference computation (B=256 rollout groups, n=64 agents, D=256):
    comm = h @ W_act.T + b_act                      # (B*n, D)
    agg[b,j] = sum_i mask[i,j] * comm[b,i] / (n-1)  # mask = ones - eye
    x   = agg @ W_sum.T + b_sum
    out = relu(x @ W_head.T + b_head)

Everything before the relu is linear, so fold on host:
    Wc = W_head @ W_sum @ W_act          (256x256)
    bc = b_head + b_sum @ W_head.T + b_act @ (W_head @ W_sum).T
    out[b,j] = relu( (A @ H_b)[j] @ Wc.T + bc ),  A = (ones-eye)/(n-1)

On device (per core, 2048 rows = 16 token tiles of 128):
    stage 1 (PE): Y.T tiles [d, tok] via matmul(lhsT=H_tile[128tok,128d],
                  rhs=blockdiag(A,A)) - aggregation and transpose fused.
    stage 2 (DVE): evict Y.T PSUM banks to SBUF (cast to fp16).
    stage 3 (PE): out[tok, d_out] = Y.T.T @ Wc.T accumulated over 2 k-chunks.
    stage 4 (ACT/DVE): relu + scale + PSUM->SBUF evict.
    stage 5: per-chunk DMA store (fine-grained, starts early).

Pipelined at 2-tile (256 KiB) granularity so input DMA, PE, DVE/ACT and
output DMA all overlap; PE is kept continuously busy (one-chunk lookahead
between agg and main stages) to hold the HAM clock gate open.

Sharding: data-parallel over the B axis, 8 cores x 32 groups.
"""

from contextlib import ExitStack

import numpy as np

import concourse.bacc as bacc
import concourse.bass as bass
import concourse.tile as tile
from concourse import mybir
from concourse.bass_utils import run_bass_kernel_spmd

N_AGENTS = 64
B = 256
D = 256
N_CORES = 8
ROWS = B * N_AGENTS            # 16384
ROWS_PER_CORE = ROWS // N_CORES  # 2048
P = 128
N_TILES = ROWS_PER_CORE // P   # 16 token tiles per core
LC = 2                         # tiles per pipeline chunk (256 KiB DMA)
N_CHUNKS = N_TILES // LC       # 8
W_SCALE = 16.0  # fp16 weight prescale (power of 2; inverted exactly in relu)

_cache = {}


def _build(has_bias: bool, f16: bool = True):
    f32 = mybir.dt.float32
    mdt = mybir.dt.float16 if f16 else mybir.dt.float32
    inv_scale = 1.0 / W_SCALE if f16 else 1.0
    nc = bacc.Bacc("TRN2", target_bir_lowering=False, debug=False,
                   num_devices=N_CORES)

    h = nc.dram_tensor("h", [ROWS_PER_CORE, D], f32, kind="ExternalInput")
    wcT = nc.dram_tensor("wcT", [D, D], mdt, kind="ExternalInput")
    ablk = nc.dram_tensor("ablk", [P, P], mdt, kind="ExternalInput")
    if has_bias:
        bc = nc.dram_tensor("bc", [1, D], f32, kind="ExternalInput")
    out = nc.dram_tensor("out", [ROWS_PER_CORE, D], f32, kind="ExternalOutput")

    h_ap = h[:, :].rearrange("(n p) d -> p n d", p=P)      # [128, 16, 256]
    out_ap = out[:, :].rearrange("(n p) d -> p n d", p=P)  # [128, 16, 256]

    with tile.TileContext(nc) as tc:
        with ExitStack() as ctx:
            const = ctx.enter_context(tc.tile_pool(name="const", bufs=1))
            aggps = ctx.enter_context(
                tc.tile_pool(name="aggps", bufs=4, space="PSUM"))
            outps = ctx.enter_context(
                tc.tile_pool(name="outps", bufs=4, space="PSUM"))

            a_t = const.tile([P, P], mdt, tag="a", name="a_t")
            w_t = [const.tile([P, D], mdt, tag=f"w{k}", name=f"w_{k}")
                   for k in range(2)]
            if has_bias:
                bc_t = const.tile([P, D], f32, tag="bc", name="bc_t")

            # ---- input DMA issue: all load triggers go first on each ring,
            # alternating rings per chunk so issue overlaps transfer.
            nc.sync.dma_start(out=a_t[:], in_=ablk[:, :])
            traw = []
            for c in range(N_CHUNKS):
                t = const.tile([P, LC, D], f32, tag=f"hr{c}", name=f"hr_{c}")
                eng = nc.sync if c % 2 == 0 else nc.scalar
                eng.dma_start(out=t[:], in_=h_ap[:, c * LC:(c + 1) * LC, :])
                traw.append(t)
                if c == 0:
                    # weights right behind the first h chunk (needed by main0)
                    for k in range(2):
                        nc.scalar.dma_start(
                            out=w_t[k][:], in_=wcT[k * P:(k + 1) * P, :])
                    if has_bias:
                        bc_bcast = bass.AP(
                            tensor=bc, offset=0, ap=[[0, P], [1, D]])
                        nc.gpsimd.dma_start(out=bc_t[:], in_=bc_bcast)

            # fp16 views of h chunks (DVE cast)
            hc = [const.tile([P, LC, D], mdt, tag=f"hc{c}", name=f"hc_{c}")
                  for c in range(N_CHUNKS)] if f16 else traw

            # Y.T in SBUF: two d-chunks, each [128 d, 2048 tok]
            yt = [const.tile([P, ROWS_PER_CORE], mdt, tag=f"yt{k}",
                             name=f"yt_{k}") for k in range(2)]
            och = [const.tile([P, LC, D], f32, tag=f"oc{c}", name=f"oc_{c}")
                   for c in range(N_CHUNKS)]

            def cast(c):
                if f16:
                    nc.vector.tensor_copy(out=hc[c][:], in_=traw[c][:])

            def agg(c):
                # 2 tiles x 2 d-chunks; one accumulation bank per (c, k)
                ps = [aggps.tile([P, LC * P], f32, tag="aggps",
                                 name="agg_ps") for _ in range(2)]
                for s in range(LC):
                    for k in range(2):
                        lhsT = hc[c][:, s, k * P:(k + 1) * P]
                        nc.tensor.matmul(
                            ps[k][:, s * P:(s + 1) * P], lhsT, a_t[:],
                            start=True, stop=True)
                for k in range(2):
                    nc.vector.tensor_copy(
                        yt[k][:, c * LC * P:(c + 1) * LC * P], ps[k][:])

            def main(c):
                for s in range(LC):
                    m = c * LC + s
                    po = outps.tile([P, D], f32, tag="outps", name="po")
                    for k in range(2):
                        nc.tensor.matmul(
                            po[:], yt[k][:, m * P:(m + 1) * P], w_t[k][:],
                            start=(k == 0), stop=(k == 1))
                    dst = och[c][:, s, :]
                    if has_bias:
                        nc.vector.tensor_scalar(
                            out=dst, in0=po[:], scalar1=inv_scale,
                            scalar2=None, op0=mybir.AluOpType.mult)
                        nc.vector.tensor_tensor(
                            out=dst, in0=dst, in1=bc_t[:],
                            op=mybir.AluOpType.add)
                        nc.scalar.activation(
                            out=dst, in_=dst,
                            func=mybir.ActivationFunctionType.Relu)
                    elif m % 2 == 0:
                        nc.scalar.activation(
                            out=dst, in_=po[:],
                            func=mybir.ActivationFunctionType.Relu,
                            scale=inv_scale)
                    else:
                        nc.vector.tensor_scalar(
                            out=dst, in0=po[:], scalar1=inv_scale,
                            scalar2=0.0, op0=mybir.AluOpType.mult,
                            op1=mybir.AluOpType.max)
                (nc.sync if c % 2 == 0 else nc.scalar).dma_start(
                    out=out_ap[:, c * LC:(c + 1) * LC, :], in_=och[c][:])

            # one-chunk lookahead keeps PE busy while DVE evicts Y.T
            cast(0)
            agg(0)
            cast(1)
            agg(1)
            for c in range(N_CHUNKS - 2):
                main(c)
                cast(c + 2)
                agg(c + 2)
            main(N_CHUNKS - 2)
            main(N_CHUNKS - 1)
    nc.finalize()
    return nc


def _fold(W_act, b_act, W_sum, b_sum, W_head, b_head, f16=True):
    Wa = W_act.astype(np.float64)
    Ws = W_sum.astype(np.float64)
    Wh = W_head.astype(np.float64)
    Wc = Wh @ Ws @ Wa
    bc = (b_head.astype(np.float64)
          + b_sum.astype(np.float64) @ Wh.T
          + b_act.astype(np.float64) @ (Wh @ Ws).T)
    A = np.ones((N_AGENTS, N_AGENTS)) - np.eye(N_AGENTS)
    if f16:
        # mask stays exact 0/1 in fp16; 1/63 and the fp16-subnormal
        # prescale fold into the weights, inverted via the relu scale.
        WcT = (Wc.T / (N_AGENTS - 1) * W_SCALE).astype(np.float16)
        wdt = np.float16
    else:
        A = A / (N_AGENTS - 1)
        WcT = Wc.T.astype(np.float32)
        wdt = np.float32
    Ablk = np.zeros((P, P))
    Ablk[:N_AGENTS, :N_AGENTS] = A
    Ablk[N_AGENTS:, N_AGENTS:] = A
    return (np.ascontiguousarray(WcT), bc.astype(np.float32),
            Ablk.astype(wdt))


def kernel(hidden_state, W_act, b_act, W_sum, b_sum, W_head, b_head,
           _trace=False, _tmpdir=None):
    import os
    f16 = os.environ.get("KERNEL_F32", "0") != "1"
    h = np.ascontiguousarray(np.asarray(hidden_state, dtype=np.float32))
    WcT, bc, Ablk = _fold(np.asarray(W_act), np.asarray(b_act),
                          np.asarray(W_sum), np.asarray(b_sum),
                          np.asarray(W_head), np.asarray(b_head), f16=f16)
    has_bias = bool(np.any(bc))
    if (has_bias, f16) not in _cache:
        _cache[(has_bias, f16)] = _build(has_bias, f16=f16)
    nc = _cache[(has_bias, f16)]

    in_maps = []
    for c in range(N_CORES):
        m = {"h": h[c * ROWS_PER_CORE:(c + 1) * ROWS_PER_CORE],
             "wcT": WcT, "ablk": Ablk}
        if has_bias:
            m["bc"] = bc.reshape(1, D)
        in_maps.append(m)

    res = run_bass_kernel_spmd(
        nc, in_maps, core_ids=list(range(N_CORES)),
        trace=_trace, tmpdir=_tmpdir)
    out = np.concatenate([res.results[c]["out"] for c in range(N_CORES)],
                         axis=0)
    if _trace:
        return out, res
    return out
